# revision 9
# baseline (speedup 1.0000x reference)
"""Trainium2 Bass kernel for nn_CurveGraphic2d (curve rasterization, KNN-min).

Full inputs -> full output. Shards the 16 curves across 8 NeuronCores
(2 curves per core). The entire data-dependent computation (De Casteljau
Bezier evaluation, arc-length re-parameterization incl. jnp.interp
emulation, distance-field min-reduction and the AA epilogue) runs on
device; the host only supplies data-independent constants (iota rows,
identity) and gathers the per-core outputs.
"""

import sys

sys.path.insert(0, "/opt/trn_rl_repo")

from contextlib import ExitStack

import numpy as np

import concourse.bass as bass
import concourse.tile as tile
from concourse import mybir
from concourse.bass_utils import run_bass_kernel_spmd

F32 = mybir.dt.float32
AT = mybir.ActivationFunctionType
OP = mybir.AluOpType

N_CORES = 8
BPC = 2  # batches (curves) per core
H = W = 256
S = 64  # samples per curve
K = 4  # control points
EPS = 1e-6
DELTA = np.float32(1.0) / np.float32(63.0)  # == jnp.linspace(0,1,64) step, exact


def split_multi_waits(nc, max_waits=1):
    """neuronxcc's walrus codegen rejects instructions carrying several sync
    waits ("Too many sync wait commands"); move excess waits onto preceding
    NoOps on the same engine."""
    ctr = 0
    for f in nc.m.functions:
        for bb in f.blocks:
            out = []
            for inst in bb.instructions:
                si = inst.sync_info
                if si is not None and si.on_wait and len(si.on_wait) > max_waits:
                    waits = list(si.on_wait)
                    extra, keep = waits[:-max_waits], waits[-max_waits:]
                    for i in range(0, len(extra), max_waits):
                        nop = mybir.InstNoOp(name=f"wait_split_{ctr}")
                        ctr += 1
                        nop.engine = inst.engine
                        nop.sync_info = mybir.SyncInfo(
                            on_wait=extra[i : i + max_waits], on_update=[]
                        )
                        out.append(nop)
                    si.on_wait = keep
                out.append(inst)
            bb.instructions = out


def build_program(split=True):
    nc = bass.Bass("TRN2", target_bir_lowering=False, debug=False)

    # -------- per-core data inputs --------
    # control points * canvas_shape, flattened (b, k, c) with c = (y, x)
    ctrl = nc.dram_tensor("ctrl", [1, BPC * K * 2], F32, kind="ExternalInput").ap()
    w_in = nc.dram_tensor("w_in", [1, BPC], F32, kind="ExternalInput").ap()
    aa_in = nc.dram_tensor("aa_in", [1, BPC], F32, kind="ExternalInput").ap()

    # -------- data-independent constants --------
    c_x = nc.dram_tensor("c_x", [128, W], F32, kind="ExternalInput").ap()  # rows 0..255
    c_yc = nc.dram_tensor("c_yc", [128, 2], F32, kind="ExternalInput").ap()  # y cols
    c_ts0 = nc.dram_tensor("c_ts0", [S, 1], F32, kind="ExternalInput").ap()
    c_eye = nc.dram_tensor("c_eye", [S, S], F32, kind="ExternalInput").ap()
    c_ones = nc.dram_tensor("c_ones", [1, 128], F32, kind="ExternalInput").ap()
    c_mones = nc.dram_tensor("c_mones", [1, 128], F32, kind="ExternalInput").ap()

    out_d = nc.dram_tensor("out", [BPC, H, W], F32, kind="ExternalOutput").ap()

    with tile.TileContext(nc) as tc, ExitStack() as ctx:
        cpool = ctx.enter_context(tc.tile_pool(name="const", bufs=1))
        ppool = ctx.enter_context(tc.tile_pool(name="prep", bufs=1))
        psum = ctx.enter_context(tc.tile_pool(name="psum", bufs=1, space="PSUM"))
        sqpool = ctx.enter_context(tc.tile_pool(name="sq", bufs=6))
        accpool = ctx.enter_context(tc.tile_pool(name="acc", bufs=2))
        epool = ctx.enter_context(tc.tile_pool(name="epi", bufs=2))

        # ---- load constants & inputs ----
        xt = cpool.tile([128, W], F32, tag="xt")
        nc.sync.dma_start(xt[:], c_x[:])
        yc = cpool.tile([128, 2], F32, tag="yc")
        nc.sync.dma_start(yc[:], c_yc[:])
        ts0 = cpool.tile([S, 1], F32, tag="ts0")
        nc.sync.dma_start(ts0[:], c_ts0[:])
        eye = cpool.tile([S, S], F32, tag="eye")
        nc.sync.dma_start(eye[:], c_eye[:])
        ones = cpool.tile([1, 128], F32, tag="ones")
        nc.sync.dma_start(ones[:], c_ones[:])
        mones = cpool.tile([1, 128], F32, tag="mones")
        nc.sync.dma_start(mones[:], c_mones[:])
        ctl = cpool.tile([1, BPC * K * 2], F32, tag="ctl")
        nc.sync.dma_start(ctl[:], ctrl[:])
        wt = cpool.tile([1, BPC], F32, tag="wt")
        nc.sync.dma_start(wt[:], w_in[:])
        aat = cpool.tile([1, BPC], F32, tag="aat")
        nc.sync.dma_start(aat[:], aa_in[:])

        # ---- A1: broadcast control points across the 64 t-partitions ----
        cp_ps = psum.tile([S, BPC * K * 2], F32, tag="ps_a")
        nc.tensor.matmul(cp_ps[:], ones[:, 0:S], ctl[:], start=True, stop=True)
        cp = ppool.tile([S, BPC * K * 2], F32, tag="cp")
        nc.scalar.copy(cp[:], cp_ps[:])

        def casteljau(cur_ap, nk, t_col, tagp):
            # cur_ap viewed [S, b, nk, 2] -> [S, b, nk-1, 2]
            b_dim = cur_ap.shape[1]
            lo = cur_ap[:, :, 0 : nk - 1, :]
            hi = cur_ap[:, :, 1:nk, :]
            d = ppool.tile([S, b_dim * (nk - 1) * 2], F32, tag=f"d{tagp}{nk}")
            dv = d[:].rearrange("p (b k c) -> p b k c", b=b_dim, k=nk - 1)
            nc.vector.tensor_tensor(dv, hi, lo, OP.subtract)
            nxt = ppool.tile([S, b_dim * (nk - 1) * 2], F32, tag=f"n{tagp}{nk}")
            nv = nxt[:].rearrange("p (b k c) -> p b k c", b=b_dim, k=nk - 1)
            nc.vector.scalar_tensor_tensor(nv, dv, t_col, lo, OP.mult, OP.add)
            return nv

        # ---- A2: first De Casteljau at ts0 (both batches together) ----
        cur = cp[:].rearrange("p (b k c) -> p b k c", b=BPC, k=K)
        for nk in (4, 3, 2):
            cur = casteljau(cur, nk, ts0[:], "a")
        # pts0: [S, (b, c)] view
        pts0 = cur.rearrange("p b k c -> p (b k c)")  # [S, 4]

        # ---- A3: segment lengths ----
        # Partition-offset reads are unsupported, so transpose pts0 to a
        # single row [1, (b c, t)] first and do the diff along the free axis.
        p0T_ps = psum.tile([1, BPC * 2 * S], F32, tag="ps_b")
        pts0t = ppool.tile([S, BPC * 2], F32, tag="pts0t")
        nc.vector.tensor_copy(pts0t[:], pts0)
        for col in range(BPC * 2):
            nc.tensor.transpose(
                p0T_ps[:, col * S : (col + 1) * S], pts0t[:, col : col + 1], eye[:]
            )
        p0r = ppool.tile([1, BPC * 2 * S], F32, tag="p0r")
        nc.scalar.copy(p0r[:], p0T_ps[:])
        p0v = p0r[:].rearrange("p (b c t) -> p b c t", b=BPC, c=2)
        df = ppool.tile([1, BPC * 2 * (S - 1)], F32, tag="df")
        dfv = df[:].rearrange("p (b c t) -> p b c t", b=BPC, c=2)
        nc.vector.tensor_tensor(dfv, p0v[:, :, :, 1:S], p0v[:, :, :, 0 : S - 1], OP.subtract)
        sq4 = ppool.tile([1, BPC * 2 * (S - 1)], F32, tag="sq4")
        s4 = sq4[:].rearrange("p (b c t) -> p b c t", b=BPC, c=2)
        nc.vector.tensor_tensor(s4, dfv, dfv, OP.mult)
        seg2 = ppool.tile([1, BPC * (S - 1)], F32, tag="seg2")
        s2v = seg2[:].rearrange("p (b one t) -> p b one t", b=BPC, one=1)
        nc.vector.tensor_tensor(s2v, s4[:, :, 0:1, :], s4[:, :, 1:2, :], OP.add)
        zcol_seg = ppool.tile([1, 1], F32, tag="zcol_seg")
        nc.vector.memset(zcol_seg[:], 0.0)
        segT = ppool.tile([1, BPC * (S - 1)], F32, tag="segT")
        nc.scalar.activation(segT[:], seg2[:], AT.Sqrt, bias=zcol_seg[:])
        zeros_like = ppool.tile([1, S - 1], F32, tag="zseg")
        nc.vector.memset(zeros_like[:], 0.0)
        cumT = ppool.tile([1, BPC * (S - 1)], F32, tag="cumT")
        for b in range(BPC):
            nc.vector.tensor_tensor_scan(
                cumT[:, b * (S - 1) : (b + 1) * (S - 1)],
                segT[:, b * (S - 1) : (b + 1) * (S - 1)],
                zeros_like[:],
                0.0,
                OP.add,
                OP.add,
            )
        # u row [1, (b, S)]: u_0 = 0, u_j = cum_{j-1} / (cum_last + eps)
        clast = ppool.tile([1, BPC], F32, tag="clast")
        cl_view = cumT[:].rearrange("p (b k) -> p b k", b=BPC)[:, :, S - 2 : S - 1]
        nc.vector.tensor_scalar(
            clast[:].rearrange("p (b one) -> p b one", b=BPC),
            cl_view,
            float(EPS),
            None,
            OP.add,
        )
        crec = ppool.tile([1, BPC], F32, tag="crec")
        nc.vector.reciprocal(crec[:], clast[:])
        # broadcast crec along free via PE (outer product with a ones row)
        crb_ps = psum.tile([1, BPC * (S - 1)], F32, tag="ps_c")
        for b in range(BPC):
            nc.tensor.matmul(
                crb_ps[:, b * (S - 1) : (b + 1) * (S - 1)],
                crec[:, b : b + 1],
                ones[:, 0 : S - 1],
                start=True,
                stop=True,
            )
        u = ppool.tile([1, BPC * S], F32, tag="u")
        uv = u[:].rearrange("p (b k) -> p b k", b=BPC)
        nc.vector.memset(uv[:, :, 0:1], 0.0)
        nc.vector.tensor_tensor(
            uv[:, :, 1:S],
            cumT[:].rearrange("p (b k) -> p b k", b=BPC),
            crb_ps[:].rearrange("p (b k) -> p b k", b=BPC),
            OP.mult,
        )

        # ---- A4: broadcast u across q-partitions: Ubc [S, (b, S)] ----
        ub_ps = psum.tile([S, BPC * S], F32, tag="ps_c")
        for b in range(BPC):
            nc.tensor.matmul(
                ub_ps[:, b * S : (b + 1) * S],
                ones[:, 0:S],
                u[:, b * S : (b + 1) * S],
                start=True,
                stop=True,
            )
        ubc = ppool.tile([S, BPC * S], F32, tag="ubc")
        nc.scalar.copy(ubc[:], ub_ps[:])
        ub3 = ubc[:].rearrange("p (b k) -> p b k", b=BPC)

        # ---- A5..A13: jnp.interp(ts0, u, ts0) ----
        cmp = ppool.tile([S, BPC * S], F32, tag="cmp")
        cm3 = cmp[:].rearrange("p (b k) -> p b k", b=BPC)
        nc.vector.tensor_scalar(cmp[:], ubc[:], ts0[:], None, OP.is_le)
        cnt = ppool.tile([S, BPC], F32, tag="cnt")
        c3 = cnt[:].rearrange("p (b one) -> p b one", b=BPC)
        nc.vector.tensor_reduce(c3, cm3, mybir.AxisListType.X, OP.add)
        ind = ppool.tile([S, BPC * (S - 1)], F32, tag="ind")
        i3 = ind[:].rearrange("p (b k) -> p b k", b=BPC)
        nc.vector.tensor_tensor(i3, cm3[:, :, 0 : S - 1], cm3[:, :, 1:S], OP.subtract)
        pr1 = ppool.tile([S, BPC * (S - 1)], F32, tag="pr1")
        p13 = pr1[:].rearrange("p (b k) -> p b k", b=BPC)
        nc.vector.tensor_tensor(p13, i3, ub3[:, :, 0 : S - 1], OP.mult)
        u_at = ppool.tile([S, BPC], F32, tag="u_at")
        ua3 = u_at[:].rearrange("p (b one) -> p b one", b=BPC)
        nc.vector.tensor_reduce(ua3, p13, mybir.AxisListType.X, OP.add)
        pr2 = ppool.tile([S, BPC * (S - 1)], F32, tag="pr2")
        p23 = pr2[:].rearrange("p (b k) -> p b k", b=BPC)
        nc.vector.tensor_tensor(p23, i3, ub3[:, :, 1:S], OP.mult)
        u_nx = ppool.tile([S, BPC], F32, tag="u_nx")
        un3 = u_nx[:].rearrange("p (b one) -> p b one", b=BPC)
        nc.vector.tensor_reduce(un3, p23, mybir.AxisListType.X, OP.add)

        den = ppool.tile([S, BPC], F32, tag="den")
        nc.vector.tensor_tensor(den[:], u_nx[:], u_at[:], OP.subtract)
        cmp63 = cm3[:, :, S - 1 : S].rearrange("p b one -> p (b one)")  # [S, BPC]
        den2 = ppool.tile([S, BPC], F32, tag="den2")
        nc.vector.tensor_tensor(den2[:], den[:], cmp63, OP.add)
        rec2 = ppool.tile([S, BPC], F32, tag="rec2")
        nc.vector.reciprocal(rec2[:], den2[:])
        base = ppool.tile([S, BPC], F32, tag="base")
        nc.vector.tensor_scalar(
            base[:], cnt[:], 1.0, float(DELTA), OP.subtract, OP.mult
        )
        dnum = ppool.tile([S, BPC], F32, tag="dnum")
        nc.vector.tensor_scalar(dnum[:], u_at[:], ts0[:], None, OP.subtract)
        m2 = ppool.tile([S, BPC], F32, tag="m2")
        nc.vector.tensor_tensor(m2[:], dnum[:], rec2[:], OP.mult)
        gfac = ppool.tile([S, BPC], F32, tag="gfac")
        nc.vector.tensor_scalar(gfac[:], cmp63, 1.0, None, OP.subtract)
        m3 = ppool.tile([S, BPC], F32, tag="m3")
        nc.vector.tensor_tensor(m3[:], m2[:], gfac[:], OP.mult)
        t_arc = ppool.tile([S, BPC], F32, tag="t_arc")
        nc.vector.scalar_tensor_tensor(
            t_arc[:], m3[:], float(DELTA), base[:], OP.mult, OP.add
        )

        # ---- A14: second De Casteljau at t_arc (per batch) ----
        sp_all = ppool.tile([S, BPC * 2], F32, tag="sp_all")
        for b in range(BPC):
            cur_b = cp[:].rearrange("p (b k c) -> p b k c", b=BPC, k=K)[
                :, b : b + 1, :, :
            ]
            t_col = t_arc[:, b : b + 1]
            for nk in (4, 3, 2):
                cur_b = casteljau(cur_b, nk, t_col, f"b{b}")
            # cur_b: [S, 1, 1, 2]
            nc.vector.tensor_copy(
                sp_all[:, 2 * b : 2 * b + 2],
                cur_b.rearrange("p b k c -> p (b k c)"),
            )

        # ---- A15: transpose sample points -> one row [1, (b c, S)] ----
        spt_ps = psum.tile([1, BPC * 2 * S], F32, tag="ps_b")
        for col in range(BPC * 2):
            nc.tensor.transpose(
                spt_ps[:, col * S : (col + 1) * S], sp_all[:, col : col + 1], eye[:]
            )
        spt = ppool.tile([1, BPC * 2 * S], F32, tag="spt")
        nc.scalar.copy(spt[:], spt_ps[:])

        # ---- A16: broadcasts: negx [128, (b s)], ybc [128, (b s)] ----
        negx_ps = psum.tile([128, BPC * S], F32, tag="ps_d")
        ybc_ps = psum.tile([128, BPC * S], F32, tag="ps_e")
        for b in range(BPC):
            nc.tensor.matmul(
                negx_ps[:, b * S : (b + 1) * S],
                mones[:],
                spt[:, (2 * b + 1) * S : (2 * b + 2) * S],
                start=True,
                stop=True,
            )
            nc.tensor.matmul(
                ybc_ps[:, b * S : (b + 1) * S],
                ones[:],
                spt[:, (2 * b) * S : (2 * b + 1) * S],
                start=True,
                stop=True,
            )
        negx = ppool.tile([128, BPC * S], F32, tag="negx")
        nc.scalar.copy(negx[:], negx_ps[:])
        ybc = ppool.tile([128, BPC * S], F32, tag="ybc")
        nc.scalar.copy(ybc[:], ybc_ps[:])

        # ---- A17: dy2[(b c)][y, s] = (y_s - y)^2, y-chunk c ----
        dy2 = {}
        for b in range(BPC):
            for c in range(2):
                dfy = ppool.tile([128, S], F32, tag=f"dfy{b}{c}")
                nc.vector.tensor_scalar(
                    dfy[:],
                    ybc[:, b * S : (b + 1) * S],
                    yc[:, c : c + 1],
                    None,
                    OP.subtract,
                )
                d2 = ppool.tile([128, S], F32, tag=f"dy2{b}{c}")
                nc.vector.tensor_tensor(d2[:], dfy[:], dfy[:], OP.mult)
                dy2[(b, c)] = d2

        # ---- A18: epilogue scalars: winv and aa as [128, 1] columns ----
        winv = ppool.tile([1, BPC], F32, tag="winv")
        nc.vector.reciprocal(winv[:], wt[:])
        row4 = ppool.tile([1, 2 * BPC], F32, tag="row4")
        nc.vector.tensor_copy(row4[:, 0:BPC], winv[:])
        nc.vector.tensor_copy(row4[:, BPC : 2 * BPC], aat[:])
        wa_ps = psum.tile([128, 2 * BPC], F32, tag="ps_b")
        nc.tensor.matmul(wa_ps[:], ones[:], row4[:], start=True, stop=True)
        wa = ppool.tile([128, 2 * BPC], F32, tag="wa")
        nc.scalar.copy(wa[:], wa_ps[:])

        # constant per-partition bias columns for ACT (no const_aps registered)
        zcol = ppool.tile([128, 1], F32, tag="zcol")
        nc.vector.memset(zcol[:], 0.0)
        ecol = ppool.tile([128, 1], F32, tag="ecol")
        nc.vector.memset(ecol[:], float(EPS))
        onecol = ppool.tile([128, 1], F32, tag="onecol")
        nc.vector.memset(onecol[:], 1.0)

        # ---- B: main loop ----
        accs = {}
        for s in range(S):
            for b in range(BPC):
                sq = sqpool.tile([128, W], F32, tag="sq")
                nc.scalar.activation(
                    sq[:], xt[:], AT.Square, bias=negx[:, b * S + s : b * S + s + 1]
                )
                for c in range(2):
                    if s == 0:
                        acc = accpool.tile([128, W], F32, tag=f"acc{b}{c}")
                        nc.vector.tensor_scalar(
                            acc[:], sq[:], dy2[(b, c)][:, 0:1], None, OP.add
                        )
                    else:
                        acc = accpool.tile([128, W], F32, tag=f"acc{b}{c}")
                        nc.vector.scalar_tensor_tensor(
                            acc[:],
                            sq[:],
                            dy2[(b, c)][:, s : s + 1],
                            accs[(b, c)][:],
                            OP.add,
                            OP.min,
                        )
                    accs[(b, c)] = acc

        # ---- C: epilogue (batched per activation table) ----
        dts = {}
        for (b, c), acc in accs.items():
            d = epool.tile([128, W], F32, tag=f"ed{b}{c}")
            nc.scalar.activation(d[:], acc[:], AT.Sqrt, bias=zcol[:])
            dts[(b, c)] = d
        lns = {}
        for (b, c), d in dts.items():
            ln = epool.tile([128, W], F32, tag=f"el{b}{c}")
            nc.scalar.activation(
                ln[:], d[:], AT.Ln, bias=ecol[:], scale=wa[:, b : b + 1]
            )
            lns[(b, c)] = ln
        exs = {}
        for (b, c), ln in lns.items():
            ex = epool.tile([128, W], F32, tag=f"ee{b}{c}")
            nc.scalar.activation(ex[:], ln[:], AT.Exp, bias=zcol[:], scale=wa[:, BPC + b : BPC + b + 1])
            exs[(b, c)] = ex
        for (b, c), ex in exs.items():
            o = epool.tile([128, W], F32, tag=f"eo{b}{c}")
            nc.scalar.activation(o[:], ex[:], AT.Relu, bias=onecol[:], scale=-1.0)
            nc.sync.dma_start(out_d[b, 128 * c : 128 * (c + 1), :], o[:])

    if split:
        split_multi_waits(nc)
    return nc


def make_constants():
    xs = np.arange(W, dtype=np.float32)
    c_x = np.broadcast_to(xs, (128, W)).copy()
    c_yc = np.stack(
        [np.arange(128, dtype=np.float32), np.arange(128, 256, dtype=np.float32)],
        axis=1,
    )
    c_ts0 = (np.arange(S, dtype=np.float32) * DELTA).reshape(S, 1)
    c_eye = np.eye(S, dtype=np.float32)
    c_ones = np.ones((1, 128), dtype=np.float32)
    c_mones = -np.ones((1, 128), dtype=np.float32)
    return {
        "c_x": c_x,
        "c_yc": np.ascontiguousarray(c_yc),
        "c_ts0": c_ts0,
        "c_eye": c_eye,
        "c_ones": c_ones,
        "c_mones": c_mones,
    }


def make_in_maps(inputs, widths, aa_factors):
    consts = make_constants()
    in_maps = []
    for i in range(N_CORES):
        lo, hi = i * BPC, (i + 1) * BPC
        ctrl = (inputs[lo:hi].astype(np.float32) * np.float32(256.0)).reshape(1, -1)
        m = dict(consts)
        m["ctrl"] = np.ascontiguousarray(ctrl)
        m["w_in"] = np.ascontiguousarray(widths[lo:hi].reshape(1, BPC)).astype(
            np.float32
        )
        m["aa_in"] = np.ascontiguousarray(aa_factors[lo:hi].reshape(1, BPC)).astype(
            np.float32
        )
        in_maps.append(m)
    return in_maps


_NC_CACHE = None


def kernel(inputs, widths, aa_factors):
    global _NC_CACHE
    if _NC_CACHE is None:
        _NC_CACHE = build_program()
    nc = _NC_CACHE
    in_maps = make_in_maps(inputs, widths, aa_factors)
    res = run_bass_kernel_spmd(nc, in_maps, list(range(N_CORES)))
    outs = [res.results[i]["out"] for i in range(N_CORES)]
    return np.concatenate(outs, axis=0).astype(np.float32)
